# revision 6
# baseline (speedup 1.0000x reference)
"""Depthwise Conv3D (3x3x3, VALID, stride 1) on 8 Trainium2 NeuronCores.

Strategy: per-channel Toeplitz matmul over the H axis on TensorE.
  out[b,do,ho,wo,f] = sum_{kd,kh,kw} x[b,do+kd,ho+kh,wo+kw,f] * w[kd,kh,kw,f]
For fixed (f,kd,kw) the sum over kh is a banded [H_in=112, HO=110] Toeplitz
matrix applied along H, so one TensorE matmul (contraction over h_in on the
partition dim) handles all 3 kh taps; the 9 (kd,kw) combinations accumulate
in PSUM. Toeplitz matrices are built on the host from the tiny weight tensor.
float32r matmuls run at 1 cycle/row (vs 4 for exact fp32) when the moving
free dim is >=256; the fp32r ISA restrictions require a depth-1 moving AP
with an even element count, so the host pre-transposes x into the slab
layout [third, h, f, d*w] making the moving operand a flat 280-element
slice (7 d-planes x 40 w-columns, with junk columns at chunk boundaries
that are never copied out).

Sharding: data-parallel over (batch, D-half) -> 8 shards. Weights
(Toeplitz + bias) replicated.
"""

import sys

sys.path.insert(0, "/opt/trn_rl_repo")

from contextlib import ExitStack

import numpy as np

B, D, H, W, F = 4, 16, 112, 112, 64
DO, HO, WO = 14, 110, 110
N_CORES = 8
DO_C = 7  # output d-planes per core
DIN_C = 9  # input d-planes per core
WIN = 40  # input w columns per third
WEV = 38  # output wo columns evacuated per third
W_SPLITS = [0, 36, 72]  # wo/w start of each third (spans overlap; same values)
FLAT = DIN_C * WIN  # 360
FLATP = 368  # padded flat (d,w) extent per (h, f)
NMM = DO_C * WIN  # 280: moving-operand length per matmul (even, >=256)

_cached = None


def _build():
    from concourse import bacc, mybir, tile

    nc = bacc.Bacc("TRN2", target_bir_lowering=False, debug=False, num_devices=N_CORES)
    f32 = mybir.dt.float32
    f32r = mybir.dt.float32r

    x_ap = nc.dram_tensor("xp", [3, H, F, FLATP], f32r, kind="ExternalInput").ap()
    t_ap = nc.dram_tensor("toep", [F, H, 9, HO], f32r, kind="ExternalInput").ap()
    b_ap = nc.dram_tensor("biasbc", [128, F], f32, kind="ExternalInput").ap()
    o_ap = nc.dram_tensor("out", [DO_C, HO, WO, F], f32, kind="ExternalOutput").ap()

    with tile.TileContext(nc) as tc, ExitStack() as ctx:
        slab_pool = ctx.enter_context(tc.tile_pool(name="slab", bufs=1))
        toep_pool = ctx.enter_context(tc.tile_pool(name="toep", bufs=3))
        stage_pool = ctx.enter_context(tc.tile_pool(name="stage", bufs=1))
        psum_pool = ctx.enter_context(tc.tile_pool(name="psum", bufs=4, space="PSUM"))
        const_pool = ctx.enter_context(tc.tile_pool(name="const", bufs=1))

        bias_t = const_pool.tile([128, F], f32, name="bias_t")
        nc.sync.dma_start(out=bias_t[:], in_=b_ap[:])

        for it, w0 in enumerate(W_SPLITS):
            slab = slab_pool.tile([H, F, FLATP], f32r, name="slab", tag="slab")
            nc.sync.dma_start(out=slab[:], in_=x_ap[it])
            stage = stage_pool.tile([HO, DO_C, WEV, F], f32, name="stage", tag="stage")
            for f in range(F):
                toep_t = toep_pool.tile([H, 9, HO], f32r, name="toep_t", tag="toep")
                nc.sync.dma_start(out=toep_t[:], in_=t_ap[f])
                psum_t = psum_pool.tile([HO, DO_C, WIN], f32, name="psum_t", tag="ps")
                for kd in range(3):
                    for kw in range(3):
                        tap = kd * 3 + kw
                        off = kd * WIN + kw
                        nc.tensor.matmul(
                            psum_t[:],
                            lhsT=toep_t[:, tap, :],
                            rhs=slab[:, f, off : off + NMM],
                            start=(tap == 0),
                            stop=(tap == 8),
                        )
                # evacuate PSUM -> staging (dropping junk w columns), add bias
                if f % 2 == 0:
                    nc.vector.tensor_scalar_add(
                        stage[:, :, :, f],
                        psum_t[:, :, 0:WEV],
                        bias_t[0:HO, f : f + 1],
                    )
                else:
                    nc.scalar.activation(
                        stage[:, :, :, f],
                        psum_t[:, :, 0:WEV],
                        mybir.ActivationFunctionType.Identity,
                        bias=bias_t[0:HO, f : f + 1],
                    )
            for do in range(DO_C):
                nc.sync.dma_start(
                    out=o_ap[do, :, w0 : w0 + WEV, :], in_=stage[:, do]
                )

    nc.compile()
    return nc


def _toeplitz(w: np.ndarray) -> np.ndarray:
    t = np.zeros((F, H, 9, HO), np.float32)
    ho = np.arange(HO)
    for kd in range(3):
        for kh in range(3):
            for kw in range(3):
                t[:, ho + kh, kd * 3 + kw, ho] = w[kd, kh, kw, 0, :][:, None]
    return t


def _pack_x(xs: np.ndarray) -> np.ndarray:
    """[DIN_C, H, W, F] -> [3, H, F, FLATP] slab layout (third, h, f, (d, w))."""
    xp = np.zeros((3, H, F, FLATP), np.float32)
    for it, w0 in enumerate(W_SPLITS):
        chunk = xs[:, :, w0 : w0 + WIN, :]  # [d, h, w, f]
        xp[it, :, :, :FLAT] = (
            chunk.transpose(1, 3, 0, 2).reshape(H, F, FLAT)
        )
    return xp


def kernel(x: np.ndarray, w: np.ndarray, b: np.ndarray) -> np.ndarray:
    global _cached
    if _cached is None:
        _cached = _build()
    nc = _cached

    from concourse.bass_utils import run_bass_kernel_spmd

    x = np.asarray(x, np.float32)
    toep = _toeplitz(np.asarray(w, np.float32))
    bias_bc = np.tile(np.asarray(b, np.float32)[None, :], (128, 1))

    in_maps = []
    for core in range(N_CORES):
        bb, dh = divmod(core, 2)
        in_maps.append(
            {
                "xp": _pack_x(x[bb, dh * DO_C : dh * DO_C + DIN_C]),
                "toep": toep,
                "biasbc": bias_bc,
            }
        )

    res = run_bass_kernel_spmd(nc, in_maps, list(range(N_CORES)))

    out = np.empty((B, DO, HO, WO, F), np.float32)
    for core in range(N_CORES):
        bb, dh = divmod(core, 2)
        out[bb, dh * DO_C : (dh + 1) * DO_C] = res.results[core]["out"]
    return out


# revision 9
# speedup vs baseline: 673.9985x; 673.9985x over previous
"""Depthwise Conv3D (3x3x3, VALID, stride 1) on 8 Trainium2 NeuronCores.

Strategy: per-channel Toeplitz matmul over the H axis on TensorE.
  out[b,do,ho,wo,f] = sum_{kd,kh,kw} x[b,do+kd,ho+kh,wo+kw,f] * w[kd,kh,kw,f]
For fixed (f,kd,kw) the sum over kh is a banded [H_in=112, HO=110] Toeplitz
matrix applied along H, so one TensorE matmul (contraction over h_in on the
partition dim) handles all 3 kh taps; the 9 (kd,kw) combinations accumulate
in PSUM. Toeplitz matrices are built on the host from the tiny weight tensor.

float32r matmuls run at 1 cycle/row (vs 4 for exact fp32) when the moving
free dim is >=256; fp32r ISA restrictions require a depth-1 moving AP with
an even element count, so the host pre-transposes x into the slab layout
[half, h, f, d*w] making the moving operand a flat 406-element slice
(7 d-planes x 58 w-columns; junk columns at chunk boundaries are never
copied out). W is processed in two halves so the f-complete output staging
fits SBUF; the Toeplitz stream is re-read once per half. Toeplitz and x
are DMA'd in 4-channel batches so every transfer is ~1-2 MB.

Sharding: data-parallel over (batch, D-half) -> 8 shards; weights
(Toeplitz + bias) replicated.
"""

import sys

sys.path.insert(0, "/opt/trn_rl_repo")

from contextlib import ExitStack

import numpy as np

B, D, H, W, F = 4, 16, 112, 112, 64
DO, HO, WO = 14, 110, 110
N_CORES = 8
DO_C = 7  # output d-planes per core
DIN_C = 9  # input d-planes per core
WIN = 58  # input w columns per half
WEV = 56  # output wo columns evacuated per half
W_SPLITS = [0, 54]  # w start of each half (both input and output)
FLAT = DIN_C * WIN  # 522
FLATP = 528  # padded flat (d,w) extent per (h, f)
NMM = DO_C * WIN  # 406: moving-operand length per matmul (even, >=256)
FQ = 4  # channels per DMA batch

_cached = None


def _build(loop_n: int = 1):
    from concourse import bacc, mybir, tile

    nc = bacc.Bacc("TRN2", target_bir_lowering=False, debug=False, num_devices=N_CORES)
    f32 = mybir.dt.float32
    f32r = mybir.dt.float32r

    x_ap = nc.dram_tensor("xp", [2, H, F, FLATP], f32r, kind="ExternalInput").ap()
    t_ap = nc.dram_tensor(
        "toep", [F // FQ, H, FQ, 9, HO], f32r, kind="ExternalInput"
    ).ap()
    b_ap = nc.dram_tensor("biasbc", [128, F], f32, kind="ExternalInput").ap()
    o_ap = nc.dram_tensor("out", [DO_C, HO, WO, F], f32, kind="ExternalOutput").ap()

    with tile.TileContext(nc) as tc, ExitStack() as ctx:
        slab_pool = ctx.enter_context(tc.tile_pool(name="slab", bufs=3))
        toep_pool = ctx.enter_context(tc.tile_pool(name="toep", bufs=2))
        stage_pool = ctx.enter_context(tc.tile_pool(name="stage", bufs=1))
        psum_pool = ctx.enter_context(tc.tile_pool(name="psum", bufs=4, space="PSUM"))
        const_pool = ctx.enter_context(tc.tile_pool(name="const", bufs=1))

        bias_t = const_pool.tile([128, F], f32, name="bias_t")
        nc.sync.dma_start(out=bias_t[:], in_=b_ap[:])

        loop_ctx = tc.For_i(0, loop_n) if loop_n > 1 else None
        if loop_ctx is not None:
            ctx.enter_context(loop_ctx)

        for ih, w0 in enumerate(W_SPLITS):
            stage = stage_pool.tile([HO, DO_C, WEV, F], f32, name="stage", tag="stage")
            for q in range(F // FQ):
                toep_q = toep_pool.tile([H, FQ, 9, HO], f32r, name="toep_q", tag="tq")
                nc.sync.dma_start(out=toep_q[:], in_=t_ap[q])
                slab_q = slab_pool.tile([H, FQ, FLATP], f32r, name="slab_q", tag="sq")
                nc.sync.dma_start(out=slab_q[:], in_=x_ap[ih, :, q * FQ : (q + 1) * FQ, :])
                for fi in range(FQ):
                    f = q * FQ + fi
                    psum_t = psum_pool.tile(
                        [HO, DO_C, WIN], f32, name="psum_t", tag="ps"
                    )
                    for kd in range(3):
                        for kw in range(3):
                            tap = kd * 3 + kw
                            off = kd * WIN + kw
                            nc.tensor.matmul(
                                psum_t[:],
                                lhsT=toep_q[:, fi, tap, :],
                                rhs=slab_q[:, fi, off : off + NMM],
                                start=(tap == 0),
                                stop=(tap == 8),
                            )
                    # evacuate PSUM -> staging (dropping junk w cols), add bias
                    if f % 2 == 0:
                        nc.vector.tensor_scalar_add(
                            stage[:, :, :, f],
                            psum_t[:, :, 0:WEV],
                            bias_t[0:HO, f : f + 1],
                        )
                    else:
                        nc.scalar.activation(
                            stage[:, :, :, f],
                            psum_t[:, :, 0:WEV],
                            mybir.ActivationFunctionType.Identity,
                            bias=bias_t[0:HO, f : f + 1],
                        )
            for do in range(DO_C):
                nc.sync.dma_start(
                    out=o_ap[do, :, w0 : w0 + WEV, :], in_=stage[:, do]
                )

    nc.compile()
    return nc


def _toeplitz(w: np.ndarray) -> np.ndarray:
    t = np.zeros((F, H, 9, HO), np.float32)
    ho = np.arange(HO)
    for kd in range(3):
        for kh in range(3):
            for kw in range(3):
                t[:, ho + kh, kd * 3 + kw, ho] = w[kd, kh, kw, 0, :][:, None]
    # [F, H, 9, HO] -> [F//FQ, H, FQ, 9, HO] quad-batched layout
    return np.ascontiguousarray(
        t.reshape(F // FQ, FQ, H, 9 * HO).transpose(0, 2, 1, 3)
    ).reshape(F // FQ, H, FQ, 9, HO)


def _pack_x(xs: np.ndarray) -> np.ndarray:
    """[DIN_C, H, W, F] -> [2, H, F, FLATP] slab layout (half, h, f, (d, w))."""
    xp = np.zeros((2, H, F, FLATP), np.float32)
    for ih, w0 in enumerate(W_SPLITS):
        chunk = xs[:, :, w0 : w0 + WIN, :]  # [d, h, w, f]
        xp[ih, :, :, :FLAT] = chunk.transpose(1, 3, 0, 2).reshape(H, F, FLAT)
    return xp


def kernel(x: np.ndarray, w: np.ndarray, b: np.ndarray) -> np.ndarray:
    global _cached
    if _cached is None:
        _cached = _build()
    nc = _cached

    from concourse.bass_utils import run_bass_kernel_spmd

    x = np.asarray(x, np.float32)
    toep = _toeplitz(np.asarray(w, np.float32))
    bias_bc = np.tile(np.asarray(b, np.float32)[None, :], (128, 1))

    in_maps = []
    for core in range(N_CORES):
        bb, dh = divmod(core, 2)
        in_maps.append(
            {
                "xp": _pack_x(x[bb, dh * DO_C : dh * DO_C + DIN_C]),
                "toep": toep,
                "biasbc": bias_bc,
            }
        )

    res = run_bass_kernel_spmd(nc, in_maps, list(range(N_CORES)))

    out = np.empty((B, DO, HO, WO, F), np.float32)
    for core in range(N_CORES):
        bb, dh = divmod(core, 2)
        out[bb, dh * DO_C : (dh + 1) * DO_C] = res.results[core]["out"]
    return out


# revision 17
# speedup vs baseline: 22950.9176x; 34.0519x over previous
"""Depthwise Conv3D (3x3x3, VALID, stride 1) on 8 Trainium2 NeuronCores.

Strategy: per-channel Toeplitz matmul over the H axis on TensorE.
  out[b,do,ho,wo,f] = sum_{kd,kh,kw} x[b,do+kd,ho+kh,wo+kw,f] * w[kd,kh,kw,f]
For fixed (f,kd,kw) the sum over kh is a banded [H_in=112, HO=110] Toeplitz
matrix applied along H, so one TensorE matmul (contraction over h_in on the
partition dim) handles all 3 kh taps; the 9 (kd,kw) combinations accumulate
in PSUM. Toeplitz matrices are built on the host from the tiny weight tensor.

float32r matmuls run at 1 cycle/row (vs 4 for exact fp32) when the moving
free dim is >=256; fp32r ISA restrictions require a depth-1 moving AP with
an even element count, so the host pre-transposes x into the slab layout
[half, h, f, d*w] making the moving operand a flat 406-element slice
(7 d-planes x 58 w-columns; junk columns at chunk boundaries are never
copied out). W is processed in two halves so the f-complete output staging
fits SBUF; the Toeplitz stream is re-read once per half. Toeplitz and x
are DMA'd in 4-channel batches so every transfer is ~1-2 MB.

Sharding: data-parallel over (batch, D-half) -> 8 shards; weights
(Toeplitz + bias) replicated.
"""

import sys

sys.path.insert(0, "/opt/trn_rl_repo")

from contextlib import ExitStack

import numpy as np

B, D, H, W, F = 4, 16, 112, 112, 64
DO, HO, WO = 14, 110, 110
N_CORES = 8
DO_C = 7  # output d-planes per core
DIN_C = 9  # input d-planes per core
WIN = 58  # input w columns per half
WEV = 56  # output wo columns evacuated per half
W_SPLITS = [0, 54]  # w start of each half (both input and output)
FLAT = DIN_C * WIN  # 522
FLATP = 528  # padded flat (d,w) extent per (h, f)
NMM = DO_C * WIN  # 406: moving-operand length per matmul (even, >=256)
FQ = 4  # channels per DMA batch

MODE = "fp32r"  # "fp32r" (rel err ~2e-4) or "bf16" (faster, rel err ~1e-3)

_cached = None


def _build(loop_n: int = 1, mode: str | None = None):
    mode = mode or MODE
    from concourse import bacc, mybir, tile

    nc = bacc.Bacc("TRN2", target_bir_lowering=False, debug=False, num_devices=N_CORES)
    f32 = mybir.dt.float32
    mdt = mybir.dt.float32r if mode == "fp32r" else mybir.dt.bfloat16

    x_ap = nc.dram_tensor("xp", [2, H, F, FLATP], mdt, kind="ExternalInput").ap()
    t_ap = nc.dram_tensor(
        "toep", [F // FQ, H, FQ, 9, HO], mdt, kind="ExternalInput"
    ).ap()
    b_ap = nc.dram_tensor("biasbc", [128, F], f32, kind="ExternalInput").ap()
    o_ap = nc.dram_tensor("out", [DO_C, HO, WO, F], f32, kind="ExternalOutput").ap()

    with tile.TileContext(nc) as tc, ExitStack() as ctx:
        slab_pool = ctx.enter_context(tc.tile_pool(name="slab", bufs=3))
        toep_pool = ctx.enter_context(tc.tile_pool(name="toep", bufs=2))
        stage_pool = ctx.enter_context(tc.tile_pool(name="stage", bufs=1))
        psum_pool = ctx.enter_context(tc.tile_pool(name="psum", bufs=6, space="PSUM"))
        const_pool = ctx.enter_context(tc.tile_pool(name="const", bufs=1))

        bias_t = const_pool.tile([128, F], f32, name="bias_t")
        nc.sync.dma_start(out=bias_t[:], in_=b_ap[:])

        loop_ctx = tc.For_i(0, loop_n) if loop_n > 1 else None
        if loop_ctx is not None:
            ctx.enter_context(loop_ctx)

        for ih, w0 in enumerate(W_SPLITS):
            stage = stage_pool.tile([HO, DO_C, WEV, F], f32, name="stage", tag="stage")
            for q in range(F // FQ):
                # input DMAs go on the ACT HWDGE ring so they never queue
                # behind the output drain on the SP ring
                toep_q = toep_pool.tile([H, FQ, 9, HO], mdt, name="toep_q", tag="tq")
                nc.scalar.dma_start(out=toep_q[:], in_=t_ap[q])
                slab_q = slab_pool.tile([H, FQ, FLATP], mdt, name="slab_q", tag="sq")
                nc.scalar.dma_start(out=slab_q[:], in_=x_ap[ih, :, q * FQ : (q + 1) * FQ, :])
                for fi in range(FQ):
                    f = q * FQ + fi
                    psum_t = psum_pool.tile(
                        [HO, DO_C, WIN], f32, name="psum_t", tag="ps"
                    )
                    for kd in range(3):
                        for kw in range(3):
                            tap = kd * 3 + kw
                            off = kd * WIN + kw
                            nc.tensor.matmul(
                                psum_t[:],
                                lhsT=toep_q[:, fi, tap, :],
                                rhs=slab_q[:, fi, off : off + NMM],
                                start=(tap == 0),
                                stop=(tap == 8),
                            )
                    # evacuate PSUM -> staging (dropping junk w cols), add bias
                    if f % 2 == 0:
                        nc.vector.tensor_scalar_add(
                            stage[:, :, :, f],
                            psum_t[:, :, 0:WEV],
                            bias_t[0:HO, f : f + 1],
                        )
                    else:
                        nc.scalar.activation(
                            stage[:, :, :, f],
                            psum_t[:, :, 0:WEV],
                            mybir.ActivationFunctionType.Identity,
                            bias=bias_t[0:HO, f : f + 1],
                        )
            for do in range(DO_C):
                nc.sync.dma_start(
                    out=o_ap[do, :, w0 : w0 + WEV, :], in_=stage[:, do]
                )

    nc.compile()
    return nc


def _np_dt(mode: str):
    if mode == "fp32r":
        return np.float32
    import ml_dtypes

    return ml_dtypes.bfloat16


def _toeplitz(w: np.ndarray, mode: str | None = None) -> np.ndarray:
    mode = mode or MODE
    t = np.zeros((F, H, 9, HO), np.float32)
    ho = np.arange(HO)
    for kd in range(3):
        for kh in range(3):
            for kw in range(3):
                t[:, ho + kh, kd * 3 + kw, ho] = w[kd, kh, kw, 0, :][:, None]
    # [F, H, 9, HO] -> [F//FQ, H, FQ, 9, HO] quad-batched layout
    t = np.ascontiguousarray(
        t.reshape(F // FQ, FQ, H, 9 * HO).transpose(0, 2, 1, 3)
    ).reshape(F // FQ, H, FQ, 9, HO)
    return t.astype(_np_dt(mode))


def _pack_x(xs: np.ndarray, mode: str | None = None) -> np.ndarray:
    """[DIN_C, H, W, F] -> [2, H, F, FLATP] slab layout (half, h, f, (d, w))."""
    mode = mode or MODE
    xp = np.zeros((2, H, F, FLATP), _np_dt(mode))
    for ih, w0 in enumerate(W_SPLITS):
        chunk = xs[:, :, w0 : w0 + WIN, :]  # [d, h, w, f]
        xp[ih, :, :, :FLAT] = chunk.transpose(1, 3, 0, 2).reshape(H, F, FLAT)
    return xp


def kernel(x: np.ndarray, w: np.ndarray, b: np.ndarray) -> np.ndarray:
    global _cached
    if _cached is None:
        _cached = _build()
    nc = _cached

    from concourse.bass_utils import run_bass_kernel_spmd

    x = np.asarray(x, np.float32)
    toep = _toeplitz(np.asarray(w, np.float32))
    bias_bc = np.tile(np.asarray(b, np.float32)[None, :], (128, 1))

    in_maps = []
    for core in range(N_CORES):
        bb, dh = divmod(core, 2)
        in_maps.append(
            {
                "xp": _pack_x(x[bb, dh * DO_C : dh * DO_C + DIN_C]),
                "toep": toep,
                "biasbc": bias_bc,
            }
        )

    res = run_bass_kernel_spmd(nc, in_maps, list(range(N_CORES)))

    out = np.empty((B, DO, HO, WO, F), np.float32)
    for core in range(N_CORES):
        bb, dh = divmod(core, 2)
        out[bb, dh * DO_C : (dh + 1) * DO_C] = res.results[core]["out"]
    return out
